# revision 6
# baseline (speedup 1.0000x reference)
"""Two-layer LSTM (linear cell/output activations) + FC head on 8 NeuronCores.

Strategy (data-parallel over batch, per the sharding hint):
  - B=32 split across 8 cores -> B_local=4 per core; weights replicated.
  - Transposed state: h^T/c^T are [H on partitions, (k,b) on free] so the
    per-step recurrence is z^T = U^T @ h^T with U tiles stationary and all
    gate math on full-partition [128, 8] tiles.
  - The two layer recurrences are INTERLEAVED: loop iteration j runs layer-0
    on chunk j and layer-1 on chunk j-1, step by step, so each layer's serial
    chain (matmul -> sigmoid -> gate math -> h) hides the other layer's.
  - xw (input projection + bias) is batch-precomputed per chunk, then folded
    into the recurrence PSUM via ONE identity matmul per step (start=True
    preload), which runs off the critical path -- no separate add on chain.
  - Gate order (i,f,o,g) host-side so one sigmoid covers i,f,o. Gate math:
    g is copied next to the persistent c state ([g|c] tile) so
    [sig_i, sig_f] (.) [g, c] -> pairwise add -> c_new -> h = sig_o (.) c_new
    is 4 back-to-back DVE ops (no cross-engine hops between them).
  - All matmul operands bf16 (fp32 PSUM); fp8 fails the 2e-2 tolerance.
"""

import os
import numpy as np
from contextlib import ExitStack

os.environ.setdefault("MYCRO_LOCAL_CACHE", "1")

B, T, I, H, O = 32, 2048, 128, 256, 128
NCORES = 8
BL = B // NCORES          # 4 batch elements per core
CHUNK = int(os.environ.get("K_CHUNK", "128"))   # timesteps per loop body
NCH = T // CHUNK          # 32 chunks
G4 = 4 * H                # 1024 gate columns
NM = G4 // 128            # 8 gate m-tiles of 128
KT = H // 128             # 2 contraction tiles
S = KT * BL               # 8 state columns per gate block

ADD_MODE = os.environ.get("K_ADD", "imm")   # 'imm': identity-MM psum preload
STAGGER = os.environ.get("K_STAGGER", "1") == "1"

_cache = {}


def _build(tiny=False, repeat=1):
    import concourse.bacc as bacc
    import concourse.bass as bass
    import concourse.tile as tile
    import concourse.mybir as mybir

    f32 = mybir.dt.float32
    mdt = mybir.dt.bfloat16
    AF = mybir.ActivationFunctionType
    ALU = mybir.AluOpType
    ET = mybir.EngineType

    nc = bacc.Bacc("TRN2", target_bir_lowering=False, debug=False,
                   num_devices=NCORES)

    xprep_d = nc.declare_dram_parameter("xprep", [I, BL, T], mdt, isOutput=False)
    w0_d = nc.declare_dram_parameter("w0", [I, G4], mdt, isOutput=False)
    u0_d = nc.declare_dram_parameter("u0", [H, G4], mdt, isOutput=False)
    w1_d = nc.declare_dram_parameter("w1", [H, G4], mdt, isOutput=False)
    u1_d = nc.declare_dram_parameter("u1", [H, G4], mdt, isOutput=False)
    wfc_d = nc.declare_dram_parameter("wfc", [H, O], mdt, isOutput=False)
    b0t_d = nc.declare_dram_parameter("b0t", [128, NM], f32, isOutput=False)
    b1t_d = nc.declare_dram_parameter("b1t", [128, NM], f32, isOutput=False)
    bfct_d = nc.declare_dram_parameter("bfct", [128, 1], f32, isOutput=False)
    ident_d = nc.declare_dram_parameter("ident", [128, 128], mdt, isOutput=False)
    out_d = nc.declare_dram_parameter("outT", [O, BL], f32, isOutput=True)

    if tiny:
        with tile.TileContext(nc) as tc, ExitStack() as ctx:
            pool = ctx.enter_context(tc.tile_pool(name="tp", bufs=1))
            t1 = pool.tile([128, BL], mdt, tag="t1")
            t2 = pool.tile([128, BL], f32, tag="t2")
            nc.sync.dma_start(t1[:, :], xprep_d[:, :, 0])
            nc.vector.tensor_copy(t2[:, :], t1[:, :])
            nc.sync.dma_start(out_d[:, :], t2[:, :])
        nc.compile()
        return nc

    with tile.TileContext(nc) as tc, ExitStack() as ctx:
        const = ctx.enter_context(tc.tile_pool(name="const", bufs=1))
        work = ctx.enter_context(tc.tile_pool(name="work", bufs=3))
        psA = ctx.enter_context(tc.tile_pool(name="psA", bufs=3, space="PSUM"))
        psB = ctx.enter_context(tc.tile_pool(name="psB", bufs=3, space="PSUM"))
        psP = ctx.enter_context(tc.tile_pool(name="psP", bufs=2, space="PSUM"))

        # Persistent SBUF residents.
        xall = const.tile([128, BL * T], mdt, tag="xall")       # col = b*T + t
        w0 = const.tile([128, G4], mdt, tag="w0")
        u0 = [const.tile([128, G4], mdt, tag=f"u0_{k}", name=f"u0_{k}")
              for k in range(KT)]
        w1 = [const.tile([128, G4], mdt, tag=f"w1_{k}", name=f"w1_{k}")
              for k in range(KT)]
        u1 = [const.tile([128, G4], mdt, tag=f"u1_{k}", name=f"u1_{k}")
              for k in range(KT)]
        wf = [const.tile([128, O], mdt, tag=f"wf_{k}", name=f"wf_{k}")
              for k in range(KT)]
        b0t = const.tile([128, NM], f32, tag="b0t")
        b1t = const.tile([128, NM], f32, tag="b1t")
        bfct = const.tile([128, 1], f32, tag="bfct")
        ident = const.tile([128, 128], mdt, tag="ident")
        # layer-0 h chunk buffer: col = k*(BL*CHUNK) + b*CHUNK + t
        h0t = const.tile([128, S * CHUNK], mdt, tag="h0t")
        # xw chunk buffers: col = m*(BL*CHUNK) + b*CHUNK + t
        xw0t = const.tile([128, NM * BL * CHUNK], mdt, tag="xw0t")
        xw1t = const.tile([128, NM * BL * CHUNK], mdt, tag="xw1t")
        # recurrent state; G = [g-scratch | c] so c sits right after g
        h1 = const.tile([128, S], mdt, tag="h1")
        G0 = const.tile([128, 2 * S], f32, tag="G0")
        G1 = const.tile([128, 2 * S], f32, tag="G1")

        nc.sync.dma_start(xall[:, :].rearrange("p (b t) -> p b t", b=BL),
                          xprep_d[:, :, :])
        nc.sync.dma_start(w0[:, :], w0_d[:, :])
        for k in range(KT):
            sl = slice(k * 128, (k + 1) * 128)
            nc.sync.dma_start(u0[k][:, :], u0_d[sl, :])
            nc.sync.dma_start(w1[k][:, :], w1_d[sl, :])
            nc.sync.dma_start(u1[k][:, :], u1_d[sl, :])
            nc.sync.dma_start(wf[k][:, :], wfc_d[sl, :])
        nc.sync.dma_start(b0t[:, :], b0t_d[:, :])
        nc.sync.dma_start(b1t[:, :], b1t_d[:, :])
        nc.sync.dma_start(bfct[:, :], bfct_d[:, :])
        nc.sync.dma_start(ident[:, :], ident_d[:, :])

        h0t_v = h0t[:, :].rearrange("p (k b t) -> p k b t", k=KT, b=BL)
        xw0_v = xw0t[:, :].rearrange("p (m b t) -> p m b t", m=NM, b=BL)
        xw1_v = xw1t[:, :].rearrange("p (m b t) -> p m b t", m=NM, b=BL)

        def proj(dst, wts, rhss, bt):
            """dst[:, m-block] = sum_k wts[k][:,m]^T @ rhss[k] + bt[:,m]."""
            n = BL * CHUNK
            for m in range(NM):
                msl = slice(m * 128, (m + 1) * 128)
                psx = psP.tile([128, n], f32, tag="psx")
                for ki in range(len(wts)):
                    nc.tensor.matmul(psx[:, :], lhsT=wts[ki][:, msl],
                                     rhs=rhss[ki], start=(ki == 0),
                                     stop=(ki == len(wts) - 1))
                nc.scalar.activation(dst[:, m * n:(m + 1) * n], psx[:, :],
                                     AF.Identity, bias=bt[:, m:m + 1])

        def lstm_step(uw, xw_v, tl, G, h1_or_none, zpool, ztag, h_out):
            """One recurrence step.  Gate cols (i,f,o,g) after host perm.
            h_out: AP [128, k, b] (layer0 h0t view) or [128, 2S] (layer1)."""
            zp = zpool.tile([128, NM * BL], f32, tag=ztag)
            if h1_or_none is None:
                pv = (tl - 1) % CHUNK
                h_rhs = lambda k: h0t_v[:, k, :, pv]
            else:
                h_rhs = lambda k: h1_or_none[:, k * BL:(k + 1) * BL]
            if ADD_MODE == "imm":
                # preload xw into psum: zp = I^T @ xw_t  (one matmul, N=32).
                # PSUM group tracking is zero-region (2KB bank) granular, so
                # the whole tile is ONE start/stop group: stop only on the
                # very last accumulating matmul.
                nc.tensor.matmul(
                    zp[:, :].rearrange("p (m b) -> p m b", m=NM),
                    lhsT=ident[:, :], rhs=xw_v[:, :, :, tl],
                    start=True, stop=False)
                for m in range(NM):
                    msl = slice(m * 128, (m + 1) * 128)
                    for k in range(KT):
                        nc.tensor.matmul(zp[:, m * BL:(m + 1) * BL],
                                         lhsT=uw[k][:, msl], rhs=h_rhs(k),
                                         start=False,
                                         stop=(m == NM - 1 and k == KT - 1))
                zsrc = zp
            else:
                for m in range(NM):
                    msl = slice(m * 128, (m + 1) * 128)
                    for k in range(KT):
                        nc.tensor.matmul(zp[:, m * BL:(m + 1) * BL],
                                         lhsT=uw[k][:, msl], rhs=h_rhs(k),
                                         start=(k == 0), stop=(k == KT - 1))
                zs32 = work.tile([128, 4 * S], f32, tag="zs32")
                nc.vector.tensor_tensor(
                    zs32[:, :].rearrange("p (m b) -> p m b", m=NM),
                    zp[:, :].rearrange("p (m b) -> p m b", m=NM),
                    xw_v[:, :, :, tl], ALU.add)
                zsrc = zs32
            # sigmoid over i,f,o
            zs = work.tile([128, 3 * S], f32, tag="zs")
            nc.scalar.activation(zs[:, :], zsrc[:, 0:3 * S], AF.Sigmoid)
            # g next to c
            nc.vector.tensor_copy(G[:, 0:S], zsrc[:, 3 * S:4 * S])
            # P = [sig_i, sig_f] * [g, c]
            P = work.tile([128, 2 * S], f32, tag="P")
            nc.vector.tensor_tensor(P[:, :], zs[:, 0:2 * S], G[:, 0:2 * S],
                                    ALU.mult)
            # c_new = i*g + f*c  (written into the c slot of G)
            nc.vector.tensor_tensor(G[:, S:2 * S], P[:, 0:S], P[:, S:2 * S],
                                    ALU.add)
            # h = sig_o * c_new
            nc.vector.tensor_tensor(
                h_out,
                zs[:, 2 * S:3 * S].rearrange("p (k b) -> p k b", k=KT),
                G[:, S:2 * S].rearrange("p (k b) -> p k b", k=KT), ALU.mult)

        def l0_step(tl):
            lstm_step(u0, xw0_v, tl, G0, None, psA, "zp0",
                      h0t_v[:, :, :, tl])

        def l1_step(tl):
            lstm_step(u1, xw1_v, tl, G1, h1, psB, "zp1",
                      h1[:, :].rearrange("p (k b) -> p k b", k=KT))

        def stage_xq(col0):
            xq = work.tile([128, BL * CHUNK], mdt, tag="xq")
            nc.vector.tensor_copy(
                xq[:, :].rearrange("p (b t) -> p b t", b=BL),
                xall[:, :].rearrange("p (b t) -> p b t",
                                     b=BL)[:, :, col0])
            return xq

        def xw1_proj():
            proj(xw1t, w1,
                 [h0t[:, k * BL * CHUNK:(k + 1) * BL * CHUNK]
                  for k in range(KT)], b1t)

        def whole_net():
            nc.vector.memset(h0t[:, :], 0.0)
            nc.vector.memset(h1[:, :], 0.0)
            nc.vector.memset(G0[:, :], 0.0)
            nc.vector.memset(G1[:, :], 0.0)

            # ---- peel chunk 0: layer 0 only ----
            xq = stage_xq(slice(0, CHUNK))
            proj(xw0t, [w0], [xq[:, :]], b0t)
            for tl in range(CHUNK):
                l0_step(tl)
            xw1_proj()

            # ---- main loop: L0 chunk j, L1 chunk j-1 ----
            with tc.For_i(CHUNK, T, CHUNK, staggered_reset=STAGGER,
                          hint_engines=(ET.PE, ET.DVE, ET.Activation)) as iv:
                xq = stage_xq(bass.ds(iv, CHUNK))
                proj(xw0t, [w0], [xq[:, :]], b0t)
                for tl in range(CHUNK):
                    l0_step(tl)
                    l1_step(tl)
                xw1_proj()

            # ---- epilogue: L1 last chunk + FC head ----
            for tl in range(CHUNK):
                l1_step(tl)

            psf = psP.tile([128, BL], f32, tag="psx")
            for k in range(KT):
                nc.tensor.matmul(psf[:, 0:BL], lhsT=wf[k][:, :],
                                 rhs=h1[:, k * BL:(k + 1) * BL],
                                 start=(k == 0), stop=(k == KT - 1))
            oT = work.tile([128, BL], f32, tag="oT")
            nc.scalar.activation(oT[:, :], psf[:, 0:BL], AF.Identity,
                                 bias=bfct[:, 0:1])
            nc.sync.dma_start(out_d[:, :], oT[:, :])

        if repeat == 1:
            whole_net()
        else:
            with tc.For_i(0, repeat, 1):
                whole_net()

    nc.compile()
    return nc


def _get_compiled():
    if "main" not in _cache:
        _cache["main"] = _build()
    return _cache["main"]


def _in_maps(input_seq, W0, U0, b0, W1, U1, b1, Wfc, bfc):
    import ml_dtypes
    mdt = ml_dtypes.bfloat16
    x = np.asarray(input_seq, dtype=np.float32)
    # reorder gate blocks (i,f,g,o) -> (i,f,o,g) so one sigmoid instr
    # covers the first three
    perm = np.concatenate([np.arange(0, 2 * H),
                           np.arange(3 * H, 4 * H),
                           np.arange(2 * H, 3 * H)])

    def gp(w):
        return np.ascontiguousarray(
            np.asarray(w, np.float32)[..., perm].astype(mdt))

    shared = {
        "w0": gp(W0),
        "u0": gp(U0),
        "w1": gp(W1),
        "u1": gp(U1),
        "wfc": np.ascontiguousarray(np.asarray(Wfc, np.float32).astype(mdt)),
        "b0t": np.ascontiguousarray(
            np.asarray(b0, np.float32)[perm].reshape(NM, 128).T),
        "b1t": np.ascontiguousarray(
            np.asarray(b1, np.float32)[perm].reshape(NM, 128).T),
        "bfct": np.ascontiguousarray(np.asarray(bfc, np.float32).reshape(1, 128).T),
        "ident": np.eye(128, dtype=mdt),
    }
    in_maps = []
    for c in range(NCORES):
        xs = x[c * BL:(c + 1) * BL]                       # [BL, T, I]
        xp = np.ascontiguousarray(xs.transpose(2, 0, 1).astype(mdt))
        m = dict(shared)
        m["xprep"] = xp
        in_maps.append(m)
    return in_maps


def _run(nc, inputs, in_maps=None):
    from concourse.bass_utils import run_bass_kernel_spmd
    if in_maps is None:
        in_maps = _in_maps(**inputs)
    res = run_bass_kernel_spmd(nc, in_maps, list(range(NCORES)))
    out = np.empty((B, 1, O), np.float32)
    for c in range(NCORES):
        out[c * BL:(c + 1) * BL, 0, :] = res.results[c]["outT"].T
    return out


def kernel(input_seq, W0, U0, b0, W1, U1, b1, Wfc, bfc):
    nc = _get_compiled()
    return _run(nc, dict(input_seq=input_seq, W0=W0, U0=U0, b0=b0, W1=W1,
                         U1=U1, b1=b1, Wfc=Wfc, bfc=bfc))


# revision 9
# speedup vs baseline: 1.6130x; 1.6130x over previous
"""Two-layer LSTM (linear cell/output activations) + FC head on 8 NeuronCores.

Strategy (data-parallel over batch, per the sharding hint):
  - B=32 split across 8 cores -> B_local=4 per core; weights replicated.
  - Transposed state: h^T/c^T are [H on partitions, (k,b) on free] so the
    per-step recurrence is z^T = U^T @ h^T with U tiles stationary and all
    gate math on full-partition [128, 8] tiles.
  - The two layer recurrences are INTERLEAVED: loop iteration j runs layer-0
    on chunk j and layer-1 on chunk j-1, step by step, so each layer's serial
    chain (matmul -> sigmoid -> gate math -> h) hides the other layer's.
  - xw (input projection + bias) is batch-precomputed per chunk, then folded
    into the recurrence PSUM via ONE identity matmul per step (start=True
    preload), which runs off the critical path -- no separate add on chain.
  - Gate order (i,f,o,g) host-side so one sigmoid covers i,f,o. Gate math:
    g is copied next to the persistent c state ([g|c] tile) so
    [sig_i, sig_f] (.) [g, c] -> pairwise add -> c_new -> h = sig_o (.) c_new
    is 4 back-to-back DVE ops (no cross-engine hops between them).
  - All matmul operands bf16 (fp32 PSUM); fp8 fails the 2e-2 tolerance.
"""

import os
import numpy as np
from contextlib import ExitStack

os.environ.setdefault("MYCRO_LOCAL_CACHE", "1")

B, T, I, H, O = 32, 2048, 128, 256, 128
NCORES = 8
BL = B // NCORES          # 4 batch elements per core
CHUNK = int(os.environ.get("K_CHUNK", "128"))   # timesteps per loop body
NCH = T // CHUNK          # 32 chunks
G4 = 4 * H                # 1024 gate columns
NM = G4 // 128            # 8 gate m-tiles of 128
KT = H // 128             # 2 contraction tiles
S = KT * BL               # 8 state columns per gate block

ADD_MODE = os.environ.get("K_ADD", "imm")   # 'imm': identity-MM psum preload
STAGGER = os.environ.get("K_STAGGER", "1") == "1"
# SPLIT: gate order (i,f,g,o), z psum split into if/g/o tiles so the i,f
# sigmoid starts after half the matmuls and o's sigmoid runs in parallel
# with the c update.  Implies ADD_MODE='imm'.
SPLIT = os.environ.get("K_SPLIT", "0") == "1"

_cache = {}


def _build(tiny=False, repeat=1):
    import concourse.bacc as bacc
    import concourse.bass as bass
    import concourse.tile as tile
    import concourse.mybir as mybir

    f32 = mybir.dt.float32
    mdt = mybir.dt.bfloat16
    AF = mybir.ActivationFunctionType
    ALU = mybir.AluOpType
    ET = mybir.EngineType

    nc = bacc.Bacc("TRN2", target_bir_lowering=False, debug=False,
                   num_devices=NCORES)

    xprep_d = nc.declare_dram_parameter("xprep", [I, BL, T], mdt, isOutput=False)
    w0_d = nc.declare_dram_parameter("w0", [I, G4], mdt, isOutput=False)
    u0_d = nc.declare_dram_parameter("u0", [H, G4], mdt, isOutput=False)
    w1_d = nc.declare_dram_parameter("w1", [H, G4], mdt, isOutput=False)
    u1_d = nc.declare_dram_parameter("u1", [H, G4], mdt, isOutput=False)
    wfc_d = nc.declare_dram_parameter("wfc", [H, O], mdt, isOutput=False)
    b0t_d = nc.declare_dram_parameter("b0t", [128, NM], f32, isOutput=False)
    b1t_d = nc.declare_dram_parameter("b1t", [128, NM], f32, isOutput=False)
    bfct_d = nc.declare_dram_parameter("bfct", [128, 1], f32, isOutput=False)
    ident_d = nc.declare_dram_parameter("ident", [128, 128], mdt, isOutput=False)
    out_d = nc.declare_dram_parameter("outT", [O, BL], f32, isOutput=True)

    if tiny:
        with tile.TileContext(nc) as tc, ExitStack() as ctx:
            pool = ctx.enter_context(tc.tile_pool(name="tp", bufs=1))
            t1 = pool.tile([128, BL], mdt, tag="t1")
            t2 = pool.tile([128, BL], f32, tag="t2")
            nc.sync.dma_start(t1[:, :], xprep_d[:, :, 0])
            nc.vector.tensor_copy(t2[:, :], t1[:, :])
            nc.sync.dma_start(out_d[:, :], t2[:, :])
        nc.compile()
        return nc

    with tile.TileContext(nc) as tc, ExitStack() as ctx:
        const = ctx.enter_context(tc.tile_pool(name="const", bufs=1))
        work = ctx.enter_context(tc.tile_pool(name="work", bufs=3))
        zbufs = 1 if SPLIT else 3
        psA = ctx.enter_context(tc.tile_pool(name="psA", bufs=zbufs, space="PSUM"))
        psB = ctx.enter_context(tc.tile_pool(name="psB", bufs=zbufs, space="PSUM"))
        psP = ctx.enter_context(tc.tile_pool(name="psP", bufs=2, space="PSUM"))

        # Persistent SBUF residents.
        xall = const.tile([128, BL * T], mdt, tag="xall")       # col = b*T + t
        w0 = const.tile([128, G4], mdt, tag="w0")
        u0 = [const.tile([128, G4], mdt, tag=f"u0_{k}", name=f"u0_{k}")
              for k in range(KT)]
        w1 = [const.tile([128, G4], mdt, tag=f"w1_{k}", name=f"w1_{k}")
              for k in range(KT)]
        u1 = [const.tile([128, G4], mdt, tag=f"u1_{k}", name=f"u1_{k}")
              for k in range(KT)]
        wf = [const.tile([128, O], mdt, tag=f"wf_{k}", name=f"wf_{k}")
              for k in range(KT)]
        b0t = const.tile([128, NM], f32, tag="b0t")
        b1t = const.tile([128, NM], f32, tag="b1t")
        bfct = const.tile([128, 1], f32, tag="bfct")
        ident = const.tile([128, 128], mdt, tag="ident")
        # layer-0 h chunk buffer: col = k*(BL*CHUNK) + b*CHUNK + t
        h0t = const.tile([128, S * CHUNK], mdt, tag="h0t")
        # xw chunk buffers: col = m*(BL*CHUNK) + b*CHUNK + t
        xw0t = const.tile([128, NM * BL * CHUNK], mdt, tag="xw0t")
        xw1t = const.tile([128, NM * BL * CHUNK], mdt, tag="xw1t")
        # recurrent state; G = [g-scratch | c] so c sits right after g
        h1 = const.tile([128, S], mdt, tag="h1")
        G0 = const.tile([128, 2 * S], f32, tag="G0")
        G1 = const.tile([128, 2 * S], f32, tag="G1")

        nc.sync.dma_start(xall[:, :].rearrange("p (b t) -> p b t", b=BL),
                          xprep_d[:, :, :])
        nc.sync.dma_start(w0[:, :], w0_d[:, :])
        for k in range(KT):
            sl = slice(k * 128, (k + 1) * 128)
            nc.sync.dma_start(u0[k][:, :], u0_d[sl, :])
            nc.sync.dma_start(w1[k][:, :], w1_d[sl, :])
            nc.sync.dma_start(u1[k][:, :], u1_d[sl, :])
            nc.sync.dma_start(wf[k][:, :], wfc_d[sl, :])
        nc.sync.dma_start(b0t[:, :], b0t_d[:, :])
        nc.sync.dma_start(b1t[:, :], b1t_d[:, :])
        nc.sync.dma_start(bfct[:, :], bfct_d[:, :])
        nc.sync.dma_start(ident[:, :], ident_d[:, :])

        h0t_v = h0t[:, :].rearrange("p (k b t) -> p k b t", k=KT, b=BL)
        xw0_v = xw0t[:, :].rearrange("p (m b t) -> p m b t", m=NM, b=BL)
        xw1_v = xw1t[:, :].rearrange("p (m b t) -> p m b t", m=NM, b=BL)

        def proj(dst, wts, rhss, bt):
            """dst[:, m-block] = sum_k wts[k][:,m]^T @ rhss[k] + bt[:,m]."""
            n = BL * CHUNK
            for m in range(NM):
                msl = slice(m * 128, (m + 1) * 128)
                psx = psP.tile([128, n], f32, tag="psx")
                for ki in range(len(wts)):
                    nc.tensor.matmul(psx[:, :], lhsT=wts[ki][:, msl],
                                     rhs=rhss[ki], start=(ki == 0),
                                     stop=(ki == len(wts) - 1))
                nc.scalar.activation(dst[:, m * n:(m + 1) * n], psx[:, :],
                                     AF.Identity, bias=bt[:, m:m + 1])

        def lstm_step_split(uw, xw_v, tl, G, h1_or_none, zpool, ztag, h_out):
            """Split-z step: gate order (i,f,g,o), z in 3 psum tiles so the
            i,f sigmoid starts after 8 of 16 U-matmuls and sigma(o) overlaps
            the c update."""
            zp_if = zpool.tile([128, 2 * S], f32, tag=ztag + "if")
            zp_g = zpool.tile([128, S], f32, tag=ztag + "g")
            zp_o = zpool.tile([128, S], f32, tag=ztag + "o")
            if h1_or_none is None:
                pv = (tl - 1) % CHUNK
                h_rhs = lambda k: h0t_v[:, k, :, pv]
            else:
                h_rhs = lambda k: h1_or_none[:, k * BL:(k + 1) * BL]
            nc.tensor.matmul(zp_if[:, :].rearrange("p (m b) -> p m b", m=4),
                             lhsT=ident[:, :], rhs=xw_v[:, 0:4, :, tl],
                             start=True, stop=False)
            nc.tensor.matmul(zp_g[:, :].rearrange("p (m b) -> p m b", m=2),
                             lhsT=ident[:, :], rhs=xw_v[:, 4:6, :, tl],
                             start=True, stop=False)
            nc.tensor.matmul(zp_o[:, :].rearrange("p (m b) -> p m b", m=2),
                             lhsT=ident[:, :], rhs=xw_v[:, 6:8, :, tl],
                             start=True, stop=False)
            for m in range(NM):
                msl = slice(m * 128, (m + 1) * 128)
                dst, base = ((zp_if, 0) if m < 4 else
                             (zp_g, 4) if m < 6 else (zp_o, 6))
                for k in range(KT):
                    nc.tensor.matmul(
                        dst[:, (m - base) * BL:(m - base + 1) * BL],
                        lhsT=uw[k][:, msl], rhs=h_rhs(k),
                        start=False,
                        stop=(k == KT - 1 and m in (3, 5, NM - 1)))
            zs = work.tile([128, 2 * S], f32, tag="zsif")
            nc.scalar.activation(zs[:, :], zp_if[:, :], AF.Sigmoid)
            nc.vector.tensor_copy(G[:, 0:S], zp_g[:, :])
            zo = work.tile([128, S], f32, tag="zo")
            nc.scalar.activation(zo[:, :], zp_o[:, :], AF.Sigmoid)
            P = work.tile([128, 2 * S], f32, tag="P")
            nc.vector.tensor_tensor(P[:, :], zs[:, :], G[:, 0:2 * S], ALU.mult)
            nc.vector.tensor_tensor(G[:, S:2 * S], P[:, 0:S], P[:, S:2 * S],
                                    ALU.add)
            nc.vector.tensor_tensor(
                h_out,
                zo[:, :].rearrange("p (k b) -> p k b", k=KT),
                G[:, S:2 * S].rearrange("p (k b) -> p k b", k=KT), ALU.mult)

        def lstm_step(uw, xw_v, tl, G, h1_or_none, zpool, ztag, h_out):
            """One recurrence step.  Gate cols (i,f,o,g) after host perm.
            h_out: AP [128, k, b] (layer0 h0t view) or [128, 2S] (layer1)."""
            if SPLIT:
                return lstm_step_split(uw, xw_v, tl, G, h1_or_none, zpool,
                                       ztag, h_out)
            zp = zpool.tile([128, NM * BL], f32, tag=ztag)
            if h1_or_none is None:
                pv = (tl - 1) % CHUNK
                h_rhs = lambda k: h0t_v[:, k, :, pv]
            else:
                h_rhs = lambda k: h1_or_none[:, k * BL:(k + 1) * BL]
            if ADD_MODE == "imm":
                # preload xw into psum: zp = I^T @ xw_t  (one matmul, N=32).
                # PSUM group tracking is zero-region (2KB bank) granular, so
                # the whole tile is ONE start/stop group: stop only on the
                # very last accumulating matmul.
                nc.tensor.matmul(
                    zp[:, :].rearrange("p (m b) -> p m b", m=NM),
                    lhsT=ident[:, :], rhs=xw_v[:, :, :, tl],
                    start=True, stop=False)
                for m in range(NM):
                    msl = slice(m * 128, (m + 1) * 128)
                    for k in range(KT):
                        nc.tensor.matmul(zp[:, m * BL:(m + 1) * BL],
                                         lhsT=uw[k][:, msl], rhs=h_rhs(k),
                                         start=False,
                                         stop=(m == NM - 1 and k == KT - 1))
                zsrc = zp
            else:
                for m in range(NM):
                    msl = slice(m * 128, (m + 1) * 128)
                    for k in range(KT):
                        nc.tensor.matmul(zp[:, m * BL:(m + 1) * BL],
                                         lhsT=uw[k][:, msl], rhs=h_rhs(k),
                                         start=(k == 0), stop=(k == KT - 1))
                zs32 = work.tile([128, 4 * S], f32, tag="zs32")
                nc.vector.tensor_tensor(
                    zs32[:, :].rearrange("p (m b) -> p m b", m=NM),
                    zp[:, :].rearrange("p (m b) -> p m b", m=NM),
                    xw_v[:, :, :, tl], ALU.add)
                zsrc = zs32
            # sigmoid over i,f,o
            zs = work.tile([128, 3 * S], f32, tag="zs")
            nc.scalar.activation(zs[:, :], zsrc[:, 0:3 * S], AF.Sigmoid)
            # g next to c
            nc.vector.tensor_copy(G[:, 0:S], zsrc[:, 3 * S:4 * S])
            # P = [sig_i, sig_f] * [g, c]
            P = work.tile([128, 2 * S], f32, tag="P")
            nc.vector.tensor_tensor(P[:, :], zs[:, 0:2 * S], G[:, 0:2 * S],
                                    ALU.mult)
            # c_new = i*g + f*c  (written into the c slot of G)
            nc.vector.tensor_tensor(G[:, S:2 * S], P[:, 0:S], P[:, S:2 * S],
                                    ALU.add)
            # h = sig_o * c_new
            nc.vector.tensor_tensor(
                h_out,
                zs[:, 2 * S:3 * S].rearrange("p (k b) -> p k b", k=KT),
                G[:, S:2 * S].rearrange("p (k b) -> p k b", k=KT), ALU.mult)

        def l0_step(tl):
            lstm_step(u0, xw0_v, tl, G0, None, psA, "zp0",
                      h0t_v[:, :, :, tl])

        def l1_step(tl):
            lstm_step(u1, xw1_v, tl, G1, h1, psB, "zp1",
                      h1[:, :].rearrange("p (k b) -> p k b", k=KT))

        def stage_xq(col0):
            xq = work.tile([128, BL * CHUNK], mdt, tag="xq")
            nc.vector.tensor_copy(
                xq[:, :].rearrange("p (b t) -> p b t", b=BL),
                xall[:, :].rearrange("p (b t) -> p b t",
                                     b=BL)[:, :, col0])
            return xq

        def xw1_proj():
            proj(xw1t, w1,
                 [h0t[:, k * BL * CHUNK:(k + 1) * BL * CHUNK]
                  for k in range(KT)], b1t)

        def whole_net():
            nc.vector.memset(h0t[:, :], 0.0)
            nc.vector.memset(h1[:, :], 0.0)
            nc.vector.memset(G0[:, :], 0.0)
            nc.vector.memset(G1[:, :], 0.0)

            # ---- peel chunk 0: layer 0 only ----
            xq = stage_xq(slice(0, CHUNK))
            proj(xw0t, [w0], [xq[:, :]], b0t)
            for tl in range(CHUNK):
                l0_step(tl)
            xw1_proj()

            # ---- main loop: L0 chunk j, L1 chunk j-1 ----
            with tc.For_i(CHUNK, T, CHUNK, staggered_reset=STAGGER,
                          hint_engines=(ET.PE, ET.DVE, ET.Activation)) as iv:
                xq = stage_xq(bass.ds(iv, CHUNK))
                proj(xw0t, [w0], [xq[:, :]], b0t)
                for tl in range(CHUNK):
                    l0_step(tl)
                    l1_step(tl)
                xw1_proj()

            # ---- epilogue: L1 last chunk + FC head ----
            for tl in range(CHUNK):
                l1_step(tl)

            psf = psP.tile([128, BL], f32, tag="psx")
            for k in range(KT):
                nc.tensor.matmul(psf[:, 0:BL], lhsT=wf[k][:, :],
                                 rhs=h1[:, k * BL:(k + 1) * BL],
                                 start=(k == 0), stop=(k == KT - 1))
            oT = work.tile([128, BL], f32, tag="oT")
            nc.scalar.activation(oT[:, :], psf[:, 0:BL], AF.Identity,
                                 bias=bfct[:, 0:1])
            nc.sync.dma_start(out_d[:, :], oT[:, :])

        if repeat == 1:
            whole_net()
        else:
            with tc.For_i(0, repeat, 1):
                whole_net()

    nc.compile()
    return nc


def _get_compiled():
    if "main" not in _cache:
        _cache["main"] = _build()
    return _cache["main"]


def _in_maps(input_seq, W0, U0, b0, W1, U1, b1, Wfc, bfc):
    import ml_dtypes
    mdt = ml_dtypes.bfloat16
    x = np.asarray(input_seq, dtype=np.float32)
    # reorder gate blocks (i,f,g,o) -> (i,f,o,g) so one sigmoid instr
    # covers the first three
    perm = np.concatenate([np.arange(0, 2 * H),
                           np.arange(3 * H, 4 * H),
                           np.arange(2 * H, 3 * H)])

    def gp(w):
        return np.ascontiguousarray(
            np.asarray(w, np.float32)[..., perm].astype(mdt))

    shared = {
        "w0": gp(W0),
        "u0": gp(U0),
        "w1": gp(W1),
        "u1": gp(U1),
        "wfc": np.ascontiguousarray(np.asarray(Wfc, np.float32).astype(mdt)),
        "b0t": np.ascontiguousarray(
            np.asarray(b0, np.float32)[perm].reshape(NM, 128).T),
        "b1t": np.ascontiguousarray(
            np.asarray(b1, np.float32)[perm].reshape(NM, 128).T),
        "bfct": np.ascontiguousarray(np.asarray(bfc, np.float32).reshape(1, 128).T),
        "ident": np.eye(128, dtype=mdt),
    }
    in_maps = []
    for c in range(NCORES):
        xs = x[c * BL:(c + 1) * BL]                       # [BL, T, I]
        xp = np.ascontiguousarray(xs.transpose(2, 0, 1).astype(mdt))
        m = dict(shared)
        m["xprep"] = xp
        in_maps.append(m)
    return in_maps


def _run(nc, inputs, in_maps=None):
    from concourse.bass_utils import run_bass_kernel_spmd
    if in_maps is None:
        in_maps = _in_maps(**inputs)
    res = run_bass_kernel_spmd(nc, in_maps, list(range(NCORES)))
    out = np.empty((B, 1, O), np.float32)
    for c in range(NCORES):
        out[c * BL:(c + 1) * BL, 0, :] = res.results[c]["outT"].T
    return out


def kernel(input_seq, W0, U0, b0, W1, U1, b1, Wfc, bfc):
    nc = _get_compiled()
    return _run(nc, dict(input_seq=input_seq, W0=W0, U0=U0, b0=b0, W1=W1,
                         U1=U1, b1=b1, Wfc=Wfc, bfc=bfc))


# revision 10
# speedup vs baseline: 2.6646x; 1.6520x over previous
"""Two-layer LSTM (linear cell/output activations) + FC head on 8 NeuronCores.

Strategy (data-parallel over batch, per the sharding hint):
  - B=32 split across 8 cores -> B_local=4 per core; weights replicated.
  - Transposed state: h^T/c^T are [H on partitions, (k,b) on free] so the
    per-step recurrence is z^T = U^T @ h^T with U tiles stationary and all
    gate math on full-partition [128, 8] tiles.
  - The two layer recurrences are INTERLEAVED: loop iteration j runs layer-0
    on chunk j and layer-1 on chunk j-1, step by step, so each layer's serial
    chain (matmul -> sigmoid -> gate math -> h) hides the other layer's.
  - xw (input projection + bias) is batch-precomputed per chunk, then folded
    into the recurrence PSUM via ONE identity matmul per step (start=True
    preload), which runs off the critical path -- no separate add on chain.
  - Gate order (i,f,o,g) host-side so one sigmoid covers i,f,o. Gate math:
    g is copied next to the persistent c state ([g|c] tile) so
    [sig_i, sig_f] (.) [g, c] -> pairwise add -> c_new -> h = sig_o (.) c_new
    is 4 back-to-back DVE ops (no cross-engine hops between them).
  - All matmul operands bf16 (fp32 PSUM); fp8 fails the 2e-2 tolerance.
"""

import os
import numpy as np
from contextlib import ExitStack

os.environ.setdefault("MYCRO_LOCAL_CACHE", "1")

B, T, I, H, O = 32, 2048, 128, 256, 128
NCORES = 8
BL = B // NCORES          # 4 batch elements per core
CHUNK = int(os.environ.get("K_CHUNK", "128"))   # timesteps per loop body
NCH = T // CHUNK          # 32 chunks
G4 = 4 * H                # 1024 gate columns
NM = G4 // 128            # 8 gate m-tiles of 128
KT = H // 128             # 2 contraction tiles
S = KT * BL               # 8 state columns per gate block

ADD_MODE = os.environ.get("K_ADD", "imm")   # 'imm': identity-MM psum preload
STAGGER = os.environ.get("K_STAGGER", "1") == "1"
# SPLIT: gate order (i,f,g,o), z psum split into if/g/o tiles so the i,f
# sigmoid starts after half the matmuls and o's sigmoid runs in parallel
# with the c update.  Implies ADD_MODE='imm'.
SPLIT = os.environ.get("K_SPLIT", "0") == "1"

_cache = {}


def _build(tiny=False, repeat=1):
    import concourse.bacc as bacc
    import concourse.bass as bass
    import concourse.tile as tile
    import concourse.mybir as mybir

    f32 = mybir.dt.float32
    mdt = mybir.dt.bfloat16
    AF = mybir.ActivationFunctionType
    ALU = mybir.AluOpType
    ET = mybir.EngineType

    nc = bacc.Bacc("TRN2", target_bir_lowering=False, debug=False,
                   num_devices=NCORES)

    xprep_d = nc.declare_dram_parameter("xprep", [I, BL, T], mdt, isOutput=False)
    w0_d = nc.declare_dram_parameter("w0", [I, G4], mdt, isOutput=False)
    u0_d = nc.declare_dram_parameter("u0", [H, G4], mdt, isOutput=False)
    w1_d = nc.declare_dram_parameter("w1", [H, G4], mdt, isOutput=False)
    u1_d = nc.declare_dram_parameter("u1", [H, G4], mdt, isOutput=False)
    wfc_d = nc.declare_dram_parameter("wfc", [H, O], mdt, isOutput=False)
    b0t_d = nc.declare_dram_parameter("b0t", [128, NM], f32, isOutput=False)
    b1t_d = nc.declare_dram_parameter("b1t", [128, NM], f32, isOutput=False)
    bfct_d = nc.declare_dram_parameter("bfct", [128, 1], f32, isOutput=False)
    ident_d = nc.declare_dram_parameter("ident", [128, 128], mdt, isOutput=False)
    out_d = nc.declare_dram_parameter("outT", [O, BL], f32, isOutput=True)

    if tiny:
        with tile.TileContext(nc) as tc, ExitStack() as ctx:
            pool = ctx.enter_context(tc.tile_pool(name="tp", bufs=1))
            t1 = pool.tile([128, BL], mdt, tag="t1")
            t2 = pool.tile([128, BL], f32, tag="t2")
            nc.sync.dma_start(t1[:, :], xprep_d[:, :, 0])
            nc.vector.tensor_copy(t2[:, :], t1[:, :])
            nc.sync.dma_start(out_d[:, :], t2[:, :])
        nc.compile()
        return nc

    with tile.TileContext(nc) as tc, ExitStack() as ctx:
        const = ctx.enter_context(tc.tile_pool(name="const", bufs=1))
        work = ctx.enter_context(tc.tile_pool(name="work", bufs=3))
        zbufs = 1 if SPLIT else 3
        psA = ctx.enter_context(tc.tile_pool(name="psA", bufs=zbufs, space="PSUM"))
        psB = ctx.enter_context(tc.tile_pool(name="psB", bufs=zbufs, space="PSUM"))
        psP = ctx.enter_context(tc.tile_pool(name="psP", bufs=2, space="PSUM"))

        # Persistent SBUF residents.
        xall = const.tile([128, BL * T], mdt, tag="xall")       # col = b*T + t
        w0 = const.tile([128, G4], mdt, tag="w0")
        u0 = [const.tile([128, G4], mdt, tag=f"u0_{k}", name=f"u0_{k}")
              for k in range(KT)]
        w1 = [const.tile([128, G4], mdt, tag=f"w1_{k}", name=f"w1_{k}")
              for k in range(KT)]
        u1 = [const.tile([128, G4], mdt, tag=f"u1_{k}", name=f"u1_{k}")
              for k in range(KT)]
        wf = [const.tile([128, O], mdt, tag=f"wf_{k}", name=f"wf_{k}")
              for k in range(KT)]
        b0t = const.tile([128, NM], f32, tag="b0t")
        b1t = const.tile([128, NM], f32, tag="b1t")
        bfct = const.tile([128, 1], f32, tag="bfct")
        ident = const.tile([128, 128], mdt, tag="ident")
        # layer-0 h chunk buffer: col = k*(BL*CHUNK) + b*CHUNK + t
        h0t = const.tile([128, S * CHUNK], mdt, tag="h0t")
        # xw chunk buffers: col = m*(BL*CHUNK) + b*CHUNK + t
        xw0t = const.tile([128, NM * BL * CHUNK], mdt, tag="xw0t")
        xw1t = const.tile([128, NM * BL * CHUNK], mdt, tag="xw1t")
        # recurrent state; G = [g-scratch | c] so c sits right after g
        h1 = const.tile([128, S], mdt, tag="h1")
        G0 = const.tile([128, 2 * S], f32, tag="G0")
        G1 = const.tile([128, 2 * S], f32, tag="G1")

        nc.sync.dma_start(xall[:, :].rearrange("p (b t) -> p b t", b=BL),
                          xprep_d[:, :, :])
        nc.sync.dma_start(w0[:, :], w0_d[:, :])
        for k in range(KT):
            sl = slice(k * 128, (k + 1) * 128)
            nc.sync.dma_start(u0[k][:, :], u0_d[sl, :])
            nc.sync.dma_start(w1[k][:, :], w1_d[sl, :])
            nc.sync.dma_start(u1[k][:, :], u1_d[sl, :])
            nc.sync.dma_start(wf[k][:, :], wfc_d[sl, :])
        nc.sync.dma_start(b0t[:, :], b0t_d[:, :])
        nc.sync.dma_start(b1t[:, :], b1t_d[:, :])
        nc.sync.dma_start(bfct[:, :], bfct_d[:, :])
        nc.sync.dma_start(ident[:, :], ident_d[:, :])

        h0t_v = h0t[:, :].rearrange("p (k b t) -> p k b t", k=KT, b=BL)
        xw0_v = xw0t[:, :].rearrange("p (m b t) -> p m b t", m=NM, b=BL)
        xw1_v = xw1t[:, :].rearrange("p (m b t) -> p m b t", m=NM, b=BL)

        def proj(dst, wts, rhss, bt):
            """dst[:, m-block] = sum_k wts[k][:,m]^T @ rhss[k] + bt[:,m]."""
            n = BL * CHUNK
            for m in range(NM):
                msl = slice(m * 128, (m + 1) * 128)
                psx = psP.tile([128, n], f32, tag="psx")
                for ki in range(len(wts)):
                    nc.tensor.matmul(psx[:, :], lhsT=wts[ki][:, msl],
                                     rhs=rhss[ki], start=(ki == 0),
                                     stop=(ki == len(wts) - 1))
                nc.scalar.activation(dst[:, m * n:(m + 1) * n], psx[:, :],
                                     AF.Identity, bias=bt[:, m:m + 1])

        def lstm_step_split(uw, xw_v, tl, G, h1_or_none, zpool, ztag, h_out):
            """Split-z step: gate order (i,f,g,o), z in 3 psum tiles so the
            i,f sigmoid starts after 8 of 16 U-matmuls and sigma(o) overlaps
            the c update."""
            zp_if = zpool.tile([128, 2 * S], f32, tag=ztag + "if")
            zp_g = zpool.tile([128, S], f32, tag=ztag + "g")
            zp_o = zpool.tile([128, S], f32, tag=ztag + "o")
            if h1_or_none is None:
                pv = (tl - 1) % CHUNK
                h_rhs = lambda k: h0t_v[:, k, :, pv]
            else:
                h_rhs = lambda k: h1_or_none[:, k * BL:(k + 1) * BL]
            nc.tensor.matmul(zp_if[:, :].rearrange("p (m b) -> p m b", m=4),
                             lhsT=ident[:, :], rhs=xw_v[:, 0:4, :, tl],
                             start=True, stop=False)
            nc.tensor.matmul(zp_g[:, :].rearrange("p (m b) -> p m b", m=2),
                             lhsT=ident[:, :], rhs=xw_v[:, 4:6, :, tl],
                             start=True, stop=False)
            nc.tensor.matmul(zp_o[:, :].rearrange("p (m b) -> p m b", m=2),
                             lhsT=ident[:, :], rhs=xw_v[:, 6:8, :, tl],
                             start=True, stop=False)
            for m in range(NM):
                msl = slice(m * 128, (m + 1) * 128)
                dst, base = ((zp_if, 0) if m < 4 else
                             (zp_g, 4) if m < 6 else (zp_o, 6))
                for k in range(KT):
                    nc.tensor.matmul(
                        dst[:, (m - base) * BL:(m - base + 1) * BL],
                        lhsT=uw[k][:, msl], rhs=h_rhs(k),
                        start=False,
                        stop=(k == KT - 1 and m in (3, 5, NM - 1)))
            zs = work.tile([128, 2 * S], f32, tag="zsif")
            nc.scalar.activation(zs[:, :], zp_if[:, :], AF.Sigmoid)
            nc.vector.tensor_copy(G[:, 0:S], zp_g[:, :])
            zo = work.tile([128, S], f32, tag="zo")
            nc.scalar.activation(zo[:, :], zp_o[:, :], AF.Sigmoid)
            P = work.tile([128, 2 * S], f32, tag="P")
            nc.vector.tensor_tensor(P[:, :], zs[:, :], G[:, 0:2 * S], ALU.mult)
            nc.vector.tensor_tensor(G[:, S:2 * S], P[:, 0:S], P[:, S:2 * S],
                                    ALU.add)
            nc.vector.tensor_tensor(
                h_out,
                zo[:, :].rearrange("p (k b) -> p k b", k=KT),
                G[:, S:2 * S].rearrange("p (k b) -> p k b", k=KT), ALU.mult)

        def lstm_step(uw, xw_v, tl, G, h1_or_none, zpool, ztag, h_out):
            """One recurrence step.  Gate cols (i,f,o,g) after host perm.
            h_out: AP [128, k, b] (layer0 h0t view) or [128, 2S] (layer1)."""
            if SPLIT:
                return lstm_step_split(uw, xw_v, tl, G, h1_or_none, zpool,
                                       ztag, h_out)
            zp = zpool.tile([128, NM * BL], f32, tag=ztag)
            if h1_or_none is None:
                pv = (tl - 1) % CHUNK
                h_rhs = lambda k: h0t_v[:, k, :, pv]
            else:
                h_rhs = lambda k: h1_or_none[:, k * BL:(k + 1) * BL]
            if ADD_MODE == "imm":
                # preload xw into psum: zp = I^T @ xw_t  (one matmul, N=32).
                # PSUM group tracking is zero-region (2KB bank) granular, so
                # the whole tile is ONE start/stop group: stop only on the
                # very last accumulating matmul.
                nc.tensor.matmul(
                    zp[:, :].rearrange("p (m b) -> p m b", m=NM),
                    lhsT=ident[:, :], rhs=xw_v[:, :, :, tl],
                    start=True, stop=False)
                for m in range(NM):
                    msl = slice(m * 128, (m + 1) * 128)
                    for k in range(KT):
                        nc.tensor.matmul(zp[:, m * BL:(m + 1) * BL],
                                         lhsT=uw[k][:, msl], rhs=h_rhs(k),
                                         start=False,
                                         stop=(m == NM - 1 and k == KT - 1))
                zsrc = zp
            else:
                for m in range(NM):
                    msl = slice(m * 128, (m + 1) * 128)
                    for k in range(KT):
                        nc.tensor.matmul(zp[:, m * BL:(m + 1) * BL],
                                         lhsT=uw[k][:, msl], rhs=h_rhs(k),
                                         start=(k == 0), stop=(k == KT - 1))
                zs32 = work.tile([128, 4 * S], f32, tag="zs32")
                nc.vector.tensor_tensor(
                    zs32[:, :].rearrange("p (m b) -> p m b", m=NM),
                    zp[:, :].rearrange("p (m b) -> p m b", m=NM),
                    xw_v[:, :, :, tl], ALU.add)
                zsrc = zs32
            # sigmoid over i,f,o
            zs = work.tile([128, 3 * S], f32, tag="zs")
            nc.scalar.activation(zs[:, :], zsrc[:, 0:3 * S], AF.Sigmoid)
            # g next to c
            nc.vector.tensor_copy(G[:, 0:S], zsrc[:, 3 * S:4 * S])
            # P = [sig_i, sig_f] * [g, c]
            P = work.tile([128, 2 * S], f32, tag="P")
            nc.vector.tensor_tensor(P[:, :], zs[:, 0:2 * S], G[:, 0:2 * S],
                                    ALU.mult)
            # c_new = i*g + f*c  (written into the c slot of G)
            nc.vector.tensor_tensor(G[:, S:2 * S], P[:, 0:S], P[:, S:2 * S],
                                    ALU.add)
            # h = sig_o * c_new
            nc.vector.tensor_tensor(
                h_out,
                zs[:, 2 * S:3 * S].rearrange("p (k b) -> p k b", k=KT),
                G[:, S:2 * S].rearrange("p (k b) -> p k b", k=KT), ALU.mult)

        def l0_step(tl):
            lstm_step(u0, xw0_v, tl, G0, None, psA, "zp0",
                      h0t_v[:, :, :, tl])

        def l1_step(tl):
            lstm_step(u1, xw1_v, tl, G1, h1, psB, "zp1",
                      h1[:, :].rearrange("p (k b) -> p k b", k=KT))

        def stage_xq(col0):
            xq = work.tile([128, BL * CHUNK], mdt, tag="xq")
            nc.vector.tensor_copy(
                xq[:, :].rearrange("p (b t) -> p b t", b=BL),
                xall[:, :].rearrange("p (b t) -> p b t",
                                     b=BL)[:, :, col0])
            return xq

        def xw1_proj():
            proj(xw1t, w1,
                 [h0t[:, k * BL * CHUNK:(k + 1) * BL * CHUNK]
                  for k in range(KT)], b1t)

        def whole_net():
            nc.vector.memset(h0t[:, :], 0.0)
            nc.vector.memset(h1[:, :], 0.0)
            nc.vector.memset(G0[:, :], 0.0)
            nc.vector.memset(G1[:, :], 0.0)

            # ---- peel chunk 0: layer 0 only ----
            xq = stage_xq(slice(0, CHUNK))
            proj(xw0t, [w0], [xq[:, :]], b0t)
            for tl in range(CHUNK):
                l0_step(tl)
            xw1_proj()

            # ---- main loop: L0 chunk j, L1 chunk j-1 ----
            with tc.For_i(CHUNK, T, CHUNK, staggered_reset=STAGGER,
                          hint_engines=(ET.PE, ET.DVE, ET.Activation)) as iv:
                xq = stage_xq(bass.ds(iv, CHUNK))
                proj(xw0t, [w0], [xq[:, :]], b0t)
                for tl in range(CHUNK):
                    l0_step(tl)
                    l1_step(tl)
                xw1_proj()

            # ---- epilogue: L1 last chunk + FC head ----
            for tl in range(CHUNK):
                l1_step(tl)

            psf = psP.tile([128, BL], f32, tag="psx")
            for k in range(KT):
                nc.tensor.matmul(psf[:, 0:BL], lhsT=wf[k][:, :],
                                 rhs=h1[:, k * BL:(k + 1) * BL],
                                 start=(k == 0), stop=(k == KT - 1))
            oT = work.tile([128, BL], f32, tag="oT")
            nc.scalar.activation(oT[:, :], psf[:, 0:BL], AF.Identity,
                                 bias=bfct[:, 0:1])
            nc.sync.dma_start(out_d[:, :], oT[:, :])

        if repeat == 1:
            whole_net()
        else:
            with tc.For_i(0, repeat, 1):
                whole_net()

    nc.compile()
    return nc


def _get_compiled():
    if "main" not in _cache:
        _cache["main"] = _build()
    return _cache["main"]


def _in_maps(input_seq, W0, U0, b0, W1, U1, b1, Wfc, bfc):
    import ml_dtypes
    mdt = ml_dtypes.bfloat16
    x = np.asarray(input_seq, dtype=np.float32)
    if SPLIT:
        # split mode keeps the reference gate order (i,f,g,o)
        perm = np.arange(4 * H)
    else:
        # reorder gate blocks (i,f,g,o) -> (i,f,o,g) so one sigmoid instr
        # covers the first three
        perm = np.concatenate([np.arange(0, 2 * H),
                               np.arange(3 * H, 4 * H),
                               np.arange(2 * H, 3 * H)])

    def gp(w):
        return np.ascontiguousarray(
            np.asarray(w, np.float32)[..., perm].astype(mdt))

    shared = {
        "w0": gp(W0),
        "u0": gp(U0),
        "w1": gp(W1),
        "u1": gp(U1),
        "wfc": np.ascontiguousarray(np.asarray(Wfc, np.float32).astype(mdt)),
        "b0t": np.ascontiguousarray(
            np.asarray(b0, np.float32)[perm].reshape(NM, 128).T),
        "b1t": np.ascontiguousarray(
            np.asarray(b1, np.float32)[perm].reshape(NM, 128).T),
        "bfct": np.ascontiguousarray(np.asarray(bfc, np.float32).reshape(1, 128).T),
        "ident": np.eye(128, dtype=mdt),
    }
    in_maps = []
    for c in range(NCORES):
        xs = x[c * BL:(c + 1) * BL]                       # [BL, T, I]
        xp = np.ascontiguousarray(xs.transpose(2, 0, 1).astype(mdt))
        m = dict(shared)
        m["xprep"] = xp
        in_maps.append(m)
    return in_maps


def _run(nc, inputs, in_maps=None):
    from concourse.bass_utils import run_bass_kernel_spmd
    if in_maps is None:
        in_maps = _in_maps(**inputs)
    res = run_bass_kernel_spmd(nc, in_maps, list(range(NCORES)))
    out = np.empty((B, 1, O), np.float32)
    for c in range(NCORES):
        out[c * BL:(c + 1) * BL, 0, :] = res.results[c]["outT"].T
    return out


def kernel(input_seq, W0, U0, b0, W1, U1, b1, Wfc, bfc):
    nc = _get_compiled()
    return _run(nc, dict(input_seq=input_seq, W0=W0, U0=U0, b0=b0, W1=W1,
                         U1=U1, b1=b1, Wfc=Wfc, bfc=bfc))
